# revision 65
# baseline (speedup 1.0000x reference)
"""MultiHeadDiffAttention Trainium2 kernel (v2: fp16 + software pipelining).

Strategy (8 NeuronCores, SPMD):
  - Shard: batch (B=2) x head-groups (16 heads -> 4 groups of 4).
    Core c handles b = c//4, heads 4*(c%4) .. 4*(c%4)+3.
  - Differential attention folded into one 128-dim attention per head:
      q' = [q1 * scale | q2 * (-lam*scale)],  k' = [k1 | k2]
  - All matmul inputs are float16 (PE rate identical to f32r, but half
    the DMA/SBUF; rel-err ~1e-3 vs the 2e-2 gate).
  - Logits computed transposed (A^T[s,t]) so exp tiles feed O^T = V^T P^T
    directly.  Softmax denominator Z comes from accumulating the exp tiles
    on two engines (DVE + every-3rd-step on Pool, two independent
    accumulators) + a GPSIMD partition reduction -- no PE ones-matmuls.
    Output tiles are stored fp16 (host sums partials in f32).
  - Single software-pipelined instruction stream:
      K(b0,b1) -> V(b0,b1) -> Q(b0) -> attn(b0) [Q(b1) matmuls interleaved
      1/step] -> attn(b1) [out-proj(b0) interleaved 1/step] -> out-proj rest.
    Within attention, psa/exp run 3 steps ahead of the PV matmul so the PE
    never waits on ScalarE's exp; exp applies a constant logit bias of -16
    (softmax-invariant) to keep fp16 exp tiles and Z sums in range.
    Out-projection PSUM drains go through the Pool engine while used as
    fillers so the DVE (softmax-denominator accumulation) never backs up.
  - Per-core output is the head-group's slice of out @ W_proj (row-
    parallel); the host sums the 4 partials per batch element.
"""

import math

import numpy as np

B, T, E = 2, 2048, 2048
N_HEAD = 16
HD = 64                       # per-component head dim (q1/k1/q2/k2)
DV = 128                      # v head dim
SCALE = HD ** -0.5
LAMBDA_INIT = 0.8 - 0.6 * math.exp(-0.3 * (1 - 1))
P = 128
NHC = 4                       # heads per core
CQ = NHC * DV                 # 512: per-core q'/k'/v width
N_CORES = 8
NE = E // P                   # 16 contraction chunks
NS = T // P                   # 16 s chunks

_NC_CACHE = None


def _build_nc():
    import concourse.mybir as mybir
    import concourse.tile as tile
    from concourse import bacc

    from concourse import bass_isa

    f32 = mybir.dt.float32
    f16 = mybir.dt.float16
    EXP = mybir.ActivationFunctionType.Exp
    RADD = bass_isa.ReduceOp.add

    nc = bacc.Bacc("TRN2", target_bir_lowering=False, debug=False,
                   num_devices=N_CORES)
    xT = nc.dram_tensor("xT", [E, T], f16, kind="ExternalInput").ap()
    wq = nc.dram_tensor("wq", [E, CQ], f16, kind="ExternalInput").ap()
    wk = nc.dram_tensor("wk", [E, CQ], f16, kind="ExternalInput").ap()
    wv = nc.dram_tensor("wv", [E, CQ], f16, kind="ExternalInput").ap()
    wp = nc.dram_tensor("wp", [CQ, E], f16, kind="ExternalInput").ap()
    out = nc.dram_tensor("out", [T, E], f16, kind="ExternalOutput").ap()

    with tile.TileContext(nc) as tc:
        with tc.tile_pool(name="res", bufs=1) as res:
            xsb = res.tile([P, 2 * NE, 1024], f16, name="xsb")  # [e%128,(e,blk),t]
            qt = [res.tile([P, NHC, 1024], f16, name=f"qt{b}")
                  for b in range(2)]                       # Q'^T [d, h, t]
            kt = res.tile([P, NHC, T], f16, name="kt")     # K'^T [d, h, s]
            vsb = res.tile([P, NS, CQ], f16, name="vsb")   # V [s%128, sc, (h,dv)]
            ot = [res.tile([P, NHC, 1024], f16, name=f"ot{b}")
                  for b in range(2)]                       # O^T [dv, h, t]
            wpt = res.tile([P, NHC, E], f16, name="wpt")   # Wp [dv, h, e]
            nbias = res.tile([P, 1], f32, name="nbias")    # exp logit bias
            nc.vector.memset(nbias, -16.0)

            with (
                tc.tile_pool(name="pwm", bufs=1) as pwm,
                tc.tile_pool(name="pmisc", bufs=1) as pmisc,
            ):
                # ---------- Phase A: K, V, Q(b0) projections ----------
                def wmat_load(src, chunks=(8, 8)):
                    wsb = pwm.tile([P, NE, CQ], f16, name="wsb", tag="wm",
                                   bufs=2)
                    lo = 0
                    for n in chunks:
                        nc.sync.dma_start(
                            wsb[:, lo:lo + n, :],
                            src[lo * P:(lo + n) * P, :].rearrange(
                                "(ec p) c -> p ec c", p=P))
                        lo += n
                    return wsb

                # cold start: interleave wk sub-loads with x chunk loads in
                # consumption order so neither stream starves the other
                wksb = pwm.tile([P, NE, CQ], f16, name="wsb", tag="wm",
                                bufs=2)

                def wk_sub(lo, n):
                    nc.sync.dma_start(
                        wksb[:, lo:lo + n, :],
                        wk[lo * P:(lo + n) * P, :].rearrange(
                            "(ec p) c -> p ec c", p=P))

                ktv = [kt[:, :, 0:1024], kt[:, :, 1024:2048]]
                wk_sub(0, 1)
                nc.sync.dma_start(xsb[:, 0, :], xT[0:P, 0:1024])
                wk_sub(1, 1)
                nc.sync.dma_start(xsb[:, 2, :], xT[P:2 * P, 0:1024])
                # remaining wk subs fire inside the K(b0) e-loop
                wk_feed = {0: (2, 2), 1: (4, 2), 2: (6, 2), 3: (8, 4),
                           5: (12, 4)}
                pa_ps = tc.tile_pool(name="pa_ps", bufs=1, space="PSUM")
                pa_pool = pa_ps.__enter__()

                def qk_round(wsb, dst, blk, load_x, subs=1, w_feed=None):
                    """subs=4 splits into sub-rounds of 2 psum tiles so the
                    final PSUM->SBUF copies overlap earlier sub-rounds'
                    matmuls (shrinks the handoff to attention)."""
                    pairs = [(c, half) for c in range(4) for half in range(2)]
                    npair = 8 // subs
                    for sub in range(subs):
                        grp = pairs[sub * npair:(sub + 1) * npair]
                        pss = {ch: pa_pool.tile([P, 512], f32, name="pa",
                                                tag="pa", bufs=8)
                               for ch in grp}
                        for e in range(NE):
                            if w_feed and sub == 0 and e in w_feed:
                                wk_sub(*w_feed[e])
                            if load_x and sub == 0 and not (blk == 0 and e < 2):
                                nc.sync.dma_start(
                                    xsb[:, 2 * e + blk, :],
                                    xT[e * P:(e + 1) * P,
                                       blk * 1024:(blk + 1) * 1024])
                            for c, half in grp:
                                nc.tensor.matmul(
                                    pss[c, half],
                                    lhsT=wsb[:, e, c * P:(c + 1) * P],
                                    rhs=xsb[:, 2 * e + blk,
                                            half * 512:(half + 1) * 512],
                                    start=(e == 0), stop=(e == NE - 1),
                                )
                        for c, half in grp:
                            nc.vector.tensor_copy(
                                dst[:, c, half * 512:(half + 1) * 512],
                                pss[c, half])

                qk_round(wksb, ktv[0], 0, load_x=True, w_feed=wk_feed)
                wvsb = wmat_load(wv)          # prefetch V weights during K
                qk_round(wksb, ktv[1], 1, load_x=True)

                def v_round(wsb, blk):
                    psv = [pa_pool.tile([P, 512], f32, name="pa", tag="pa",
                                        bufs=8) for _ in range(8)]
                    for e in range(NE):
                        for tj in range(8):
                            nc.tensor.matmul(
                                psv[tj],
                                lhsT=xsb[:, 2 * e + blk,
                                         tj * P:(tj + 1) * P],
                                rhs=wsb[:, e, :],
                                start=(e == 0), stop=(e == NE - 1),
                            )
                    for tj in range(8):
                        nc.vector.tensor_copy(vsb[:, blk * 8 + tj, :],
                                              psv[tj])

                v_round(wvsb, 0)
                wqsb = wmat_load(wq)          # prefetch Q weights during V
                v_round(wvsb, 1)
                qk_round(wqsb, qt[0], 0, load_x=False, subs=8)
                pa_ps.__exit__(None, None, None)

                # wpt arrives during attention(b0)
                nc.sync.dma_start(
                    wpt, wp.rearrange("(ho p) o -> p ho o", p=P))

                pb_ps = tc.tile_pool(name="pb_ps", bufs=1, space="PSUM")
                pps = pb_ps.__enter__()

                # ---------- Phase B: attention, pipelined ----------
                # Filler generators: one PE matmul per attention step.
                qf_state = {"i": 0, "ps": None}

                def q_filler():
                    i = qf_state["i"]
                    if i >= 128:
                        return
                    r, e = divmod(i, NE)      # round r: psum tile (c, half)
                    c, half = divmod(r, 2)
                    if e == 0:
                        qf_state["ps"] = pps.tile([P, 512], f32, name="fl",
                                                  tag="fl", bufs=2)
                    nc.tensor.matmul(
                        qf_state["ps"],
                        lhsT=wqsb[:, e, c * P:(c + 1) * P],
                        rhs=xsb[:, 2 * e + 1, half * 512:(half + 1) * 512],
                        start=(e == 0), stop=(e == NE - 1),
                    )
                    if e == NE - 1:
                        nc.vector.tensor_copy(
                            qt[1][:, c, half * 512:(half + 1) * 512],
                            qf_state["ps"])
                    qf_state["i"] = i + 1

                df_state = {"i": 0, "ps": None, "osb": None}

                def d_unit_mm(i):
                    """i-th matmul of the out-projection stream.
                    Unit u = (tj, eo) contracts 4 heads; 64 units total
                    (32 for b0 during attn(b1), 32 in the tail).  Output
                    stores are paired: one [P, 1024] DMA per eo-pair."""
                    u, h = divmod(i, NHC)
                    tj, eo = divmod(u, 4)
                    if h == 0:
                        df_state["ps"] = pps.tile([P, 512], f32, name="fl",
                                                  tag="fl", bufs=2)
                    nc.tensor.matmul(
                        df_state["ps"],
                        lhsT=ot[tj // 8][:, h, (tj % 8) * P:
                                         (tj % 8 + 1) * P],
                        rhs=wpt[:, h, eo * 512:(eo + 1) * 512],
                        start=(h == 0), stop=(h == NHC - 1),
                    )
                    if h == NHC - 1:
                        if u >= 62:
                            osbt = pmisc.tile([P, 512], f16, name="osbt",
                                              tag="osbt", bufs=2)
                            nc.vector.tensor_copy(osbt, df_state["ps"])
                            nc.sync.dma_start(
                                out[tj * P:(tj + 1) * P,
                                    eo * 512:(eo + 1) * 512],
                                osbt)
                            return
                        if eo % 2 == 0:
                            df_state["osb"] = pmisc.tile(
                                [P, 1024], f16, name="osb", tag="osb",
                                bufs=4)
                        osb = df_state["osb"]
                        nc.vector.tensor_copy(
                            osb[:, (eo % 2) * 512:(eo % 2 + 1) * 512],
                            df_state["ps"])
                        if eo % 2 == 1:
                            nc.sync.dma_start(
                                out[tj * P:(tj + 1) * P,
                                    (eo - 1) * 512:(eo + 1) * 512],
                                osb)

                def d_filler():
                    i = df_state["i"]
                    if i >= 128:
                        return
                    d_unit_mm(i)
                    df_state["i"] = i + 1

                def attn_block(blk, filler, skip_first):
                    """Flat (h, half, s) step pipeline; psa/exp run two
                    steps ahead of the PV matmul (including across half/
                    head boundaries) so the PE never waits on ScalarE.
                    exp applies bias -16 (softmax-invariant) to keep the
                    fp16 exp tiles and Z sums in range (logits reach ~26)."""
                    t0 = blk * 1024
                    steps = [(h, half, s) for h in range(NHC)
                             for half in range(2) for s in range(NS)]
                    ets = {}
                    state = {}

                    def psaexp(i):
                        h, half, s = steps[i]
                        psa = pps.tile([P, 512], f32, name="psa",
                                       tag="psa", bufs=4)
                        nc.tensor.matmul(
                            psa,
                            lhsT=kt[:, h, s * P:(s + 1) * P],
                            rhs=qt[blk][:, h, half * 512:(half + 1) * 512],
                            start=True, stop=True,
                        )
                        ets[i] = pmisc.tile([P, 512], f16, name="et",
                                            tag="et", bufs=6)
                        nc.scalar.activation(ets[i], psa, EXP, bias=nbias)

                    # fillers deferred past (h, half) boundaries: the first
                    # steps after a boundary would stall on the previous
                    # half's normalize chain (d_filler reads ot).
                    fsched = (1,) * 16
                    psaexp(0)
                    psaexp(1)
                    psaexp(2)
                    for i, (h, half, s) in enumerate(steps):
                        if i + 3 < len(steps):
                            psaexp(i + 3)
                        if i >= skip_first:
                            for _ in range(fsched[s]):
                                filler()
                        if s == 0:
                            state[h, half] = (
                                pps.tile([P, 512], f32, name="pso",
                                         tag="pso", bufs=2),
                                pmisc.tile([P, 512], f16, name="accz",
                                           tag="accz", bufs=2),
                                pmisc.tile([P, 512], f16, name="acczb",
                                           tag="acczb", bufs=2),
                            )
                        pso, accz, acczb = state[h, half]
                        nc.tensor.matmul(
                            pso,
                            lhsT=vsb[:, s, h * P:(h + 1) * P],
                            rhs=ets[i],
                            start=(s == 0), stop=(s == NS - 1),
                        )
                        if s == 0:
                            nc.vector.tensor_copy(accz, ets[i])
                        elif s == 2:
                            nc.gpsimd.tensor_copy(acczb, ets[i])
                        elif s % 3 == 2:
                            nc.gpsimd.tensor_add(acczb, acczb, ets[i])
                        else:
                            nc.vector.tensor_add(accz, accz, ets[i])
                        del ets[i]
                        if s == NS - 1:
                            # softmax denominator + normalize
                            nc.vector.tensor_add(accz, accz, acczb)
                            zall = pmisc.tile([P, 512], f32, name="zall",
                                              tag="zall", bufs=1)
                            nc.gpsimd.partition_all_reduce(zall, accz,
                                                           channels=P,
                                                           reduce_op=RADD)
                            rb = pmisc.tile([P, 512], f32, name="rb",
                                            tag="rb", bufs=1)
                            nc.vector.reciprocal(rb, zall)
                            nc.vector.tensor_mul(
                                ot[blk][:, h, half * 512:(half + 1) * 512],
                                pso, rb)

                attn_block(0, q_filler, skip_first=0)
                attn_block(1, d_filler, skip_first=12)
                # remaining out-projection units (b0 leftovers + all of b1)
                for i in range(df_state["i"], 256):
                    d_unit_mm(i)
                pb_ps.__exit__(None, None, None)

    nc.compile()
    return nc


def _get_nc():
    global _NC_CACHE
    if _NC_CACHE is None:
        _NC_CACHE = _build_nc()
    return _NC_CACHE


def _shard_inputs(x, W_attn, W_proj, lambda_q1, lambda_k1,
                  lambda_q2, lambda_k2):
    x = np.asarray(x, np.float32)
    W_attn = np.asarray(W_attn, np.float32)
    W_proj = np.asarray(W_proj, np.float32)
    lam = float(np.exp(np.dot(np.asarray(lambda_q1, np.float32),
                              np.asarray(lambda_k1, np.float32)))
                - np.exp(np.dot(np.asarray(lambda_q2, np.float32),
                                np.asarray(lambda_k2, np.float32)))
                + LAMBDA_INIT)
    Cb = E // 2  # 1024: q1/k1/q2/k2 block width in W_attn
    in_maps = []
    for c in range(N_CORES):
        b, hg = divmod(c, 4)
        heads = [4 * hg + j for j in range(NHC)]
        wq_c = np.empty((E, CQ), np.float16)
        wk_c = np.empty((E, CQ), np.float16)
        wv_c = np.empty((E, CQ), np.float16)
        wp_c = np.empty((CQ, E), np.float16)
        for j, h in enumerate(heads):
            wq_c[:, j * P:j * P + HD] = W_attn[:, h * HD:(h + 1) * HD] * SCALE
            wq_c[:, j * P + HD:(j + 1) * P] = (
                W_attn[:, 2 * Cb + h * HD:2 * Cb + (h + 1) * HD]
                * (-lam * SCALE))
            wk_c[:, j * P:j * P + HD] = W_attn[:, Cb + h * HD:Cb + (h + 1) * HD]
            wk_c[:, j * P + HD:(j + 1) * P] = (
                W_attn[:, 3 * Cb + h * HD:3 * Cb + (h + 1) * HD])
            wv_c[:, j * P:(j + 1) * P] = (
                W_attn[:, 4 * Cb + h * DV:4 * Cb + (h + 1) * DV])
            wp_c[j * P:(j + 1) * P, :] = (
                W_proj[h * DV:(h + 1) * DV, :] * (1.0 - LAMBDA_INIT))
        in_maps.append({
            "xT": np.ascontiguousarray(x[b].T).astype(np.float16),
            "wq": wq_c, "wk": wk_c, "wv": wv_c, "wp": wp_c,
        })
    return in_maps


def _run(inputs, trace=False):
    from concourse.bass_utils import run_bass_kernel_spmd
    nc = _get_nc()
    in_maps = _shard_inputs(**inputs)
    res = run_bass_kernel_spmd(nc, in_maps, list(range(N_CORES)),
                               trace=trace)
    out = np.zeros((B, T, E), np.float32)
    for c in range(N_CORES):
        out[c // 4] += res.results[c]["out"]
    return out, res


def kernel(x, W_attn, W_proj, lambda_q1, lambda_k1, lambda_q2, lambda_k2):
    out, _ = _run(dict(x=x, W_attn=W_attn, W_proj=W_proj,
                       lambda_q1=lambda_q1, lambda_k1=lambda_k1,
                       lambda_q2=lambda_q2, lambda_k2=lambda_k2))
    return out
